# revision 21
# baseline (speedup 1.0000x reference)
"""Trainium2 Bass kernel for top-2-of-8 MoE routing (nn_MoETopX).

Reference semantics (computed densely there, routed here):
    gate_logits = x @ Wg + bg                       # [N, 8]
    top_vals, top_idx = top_k(gate_logits, 2)
    w = softmax(softmax(top_vals))                  # double softmax, [N, 2]
    h_e = x @ We[e] + be[e]       for the 2 selected experts per token
    y_e = softmax(relu(h_e), axis=-1)
    out = sum_e w_e * y_e                           # [N, 2048]

Strategy: expert-major sharding of the routed (token, expert) slot list.
The 16384 routed slots are grouped by expert into 128-row tiles and the
tiles are bin-packed onto 8 cores as S weight "segments" per core (every
core runs the identical program: same tile count T_PC and same per-segment
tile counts; which expert a segment is bound to is per-core input data).
Each core therefore loads only the S expert weight matrices its segments
need (~2x8MB bf16) instead of all 8 (64MB) -- the previous data-parallel
version was DMA-bound on exactly that weight traffic.

Per core the device program is a dense pipeline with no indirect DMA:
  1. load routed activations xg (gathered/transposed on host, bf16),
  2. per (segment, out-quarter): stream the weight block, run 16-chunk
     bf16 matmuls into PSUM for each 128-slot tile,
  3. h+bias on DVE (bias pre-broadcast to 128 partitions by the host),
     relu (+cast to bf16) on DVE, exp with fused row-sum accum on ACT,
  4. per tile: scale rows by combine_weight/rowsum, DMA out as bf16.

Host python does the routing metadata (argsort top-2, double-softmax
combine weights, bin packing, gathers/layout) and the final 2-rows-per-
token sum; all expert-matmul FLOPs (99.8% of model FLOPs) run on device.
"""

import numpy as np
import ml_dtypes

import concourse.bass as bass  # noqa: F401  (kept for parity with bass_utils expectations)
import concourse.tile as tile
from concourse import bacc, mybir
from concourse.bass_utils import run_bass_kernel_spmd

F32 = mybir.dt.float32
BF16 = mybir.dt.bfloat16

N_CORES = 8
N_TOKENS = 8192
D = 2048
O = 2048
E = 8
KC = D // 128   # 16 contraction chunks
OH = 4          # output-dim quarters (one 2KB PSUM bank per matmul)
OHW = O // OH   # 512


# ----------------------------------------------------------------------------
# Host-side routing + packing
# ----------------------------------------------------------------------------

def _softmax2(v):
    m = v.max(axis=1, keepdims=True)
    e = np.exp(v - m)
    return e / e.sum(axis=1, keepdims=True)


def _route(x, Wg, bg):
    """fp32 gate, top-2 (matches jax.lax.top_k tie order), double softmax."""
    logits = x @ Wg + bg
    order = np.argsort(-logits, axis=1, kind="stable")
    top2 = order[:, :2].astype(np.int32)
    v = np.take_along_axis(logits, top2, axis=1)
    w = _softmax2(_softmax2(v))
    return top2, w.astype(np.float32)


def _size_candidates(T_pc):
    """Per-core segment tile-count vectors to try, fewest segments first
    (fewer segments = fewer expert weight blocks DMAd per core)."""
    out = []
    for S in (1, 2, 3, 4):
        if T_pc >= S:
            base, r = divmod(T_pc, S)
            out.append(tuple([base + 1] * r + [base] * (S - r)))
    return out


def _try_assign(tiles_e, sizes):
    """Greedily assign each expert a multiset of unit sizes (units = 8 copies
    of `sizes`) covering tiles_e with minimal waste. Returns {e: {size: n}}
    or None."""
    pool = {}
    for sz in sizes:
        pool[sz] = pool.get(sz, 0) + N_CORES
    szs = sorted(pool)
    assign = {}
    for e in sorted(range(E), key=lambda e: -tiles_e[e]):
        need = int(tiles_e[e])
        if need == 0:
            assign[e] = {}
            continue
        best = None
        counts = [range(pool[s] + 1) for s in szs]
        import itertools
        for combo in itertools.product(*counts):
            tot = sum(c * s for c, s in zip(combo, szs))
            if tot < need:
                continue
            cand = (tot - need, sum(combo), combo)
            if best is None or cand[:2] < best[:2]:
                best = cand
        if best is None:
            return None
        assign[e] = {s: c for s, c in zip(szs, best[2]) if c}
        for s, c in assign[e].items():
            pool[s] -= c
    return assign


def _plan_structure(tiles_e):
    TT = int(tiles_e.sum())
    t_min = max(1, -(-TT // N_CORES))
    for T_pc in range(t_min, t_min + 8):
        for sizes in _size_candidates(T_pc):
            asg = _try_assign(tiles_e, sizes)
            if asg is not None:
                return T_pc, sizes, asg
    raise RuntimeError("packing failed")


def _wseg_of(cache, We, e):
    if e not in cache:
        cache[e] = np.ascontiguousarray(
            We[e].reshape(KC, 128, OH, OHW).transpose(2, 1, 0, 3)
        ).astype(ml_dtypes.bfloat16)
    return cache[e]


def make_plan(x, We, be, Wg, bg):
    x = np.asarray(x, dtype=np.float32)
    We = np.asarray(We, dtype=np.float32)
    be = np.asarray(be, dtype=np.float32)
    Wg = np.asarray(Wg, dtype=np.float32)
    bg = np.asarray(bg, dtype=np.float32)

    top2, w = _route(x, Wg, bg)
    cnt = np.bincount(top2.ravel(), minlength=E)
    tiles_e = np.ceil(cnt / 128).astype(int)
    T_pc, sizes, assign = _plan_structure(tiles_e)
    S = len(sizes)
    S_slots = T_pc * 128
    seg_start = np.concatenate([[0], np.cumsum(sizes)])[:-1]

    # instantiate units: per size, free (core, seg) list
    free = {}
    for c in range(N_CORES):
        for si, sz in enumerate(sizes):
            free.setdefault(sz, []).append((c, si))
    expert_units = {}
    for e in range(E):
        expert_units[e] = []
        for sz, k in sorted(assign[e].items(), reverse=True):
            for _ in range(k):
                expert_units[e].append((free[sz].pop(), sz))

    tok = np.zeros((N_CORES, S_slots), np.int32)
    wgt = np.zeros((N_CORES, S_slots), np.float32)
    seg_expert = np.zeros((N_CORES, S), np.int32)
    rows_tok, rows_gid = [], []
    for e in range(E):
        sel_t, sel_r = np.where(top2 == e)
        we_vals = w[sel_t, sel_r]
        off = 0
        for (c, si), sz in expert_units[e]:
            seg_expert[c, si] = e
            n = min(sz * 128, len(sel_t) - off)
            if n <= 0:
                continue
            base = seg_start[si] * 128
            tok[c, base:base + n] = sel_t[off:off + n]
            wgt[c, base:base + n] = we_vals[off:off + n]
            gid0 = c * S_slots + base
            rows_gid.append(np.arange(gid0, gid0 + n, dtype=np.int64))
            rows_tok.append(sel_t[off:off + n])
            off += n
        assert off == len(sel_t), (e, off, len(sel_t))

    at = np.concatenate(rows_tok)
    ag = np.concatenate(rows_gid)
    rid = ag[np.argsort(at, kind="stable")].reshape(N_TOKENS, 2)

    in_maps = []
    wcache = {}
    for c in range(N_CORES):
        A = x[tok[c]]                                     # [S_slots, D]
        XG = np.ascontiguousarray(
            A.reshape(T_pc, 128, KC, 128).transpose(3, 0, 2, 1)
        ).astype(ml_dtypes.bfloat16)                      # [128, T_pc, KC, 128]
        WSEG = np.stack([_wseg_of(wcache, We, int(seg_expert[c, si]))
                         for si in range(S)])             # [S, OH, 128, KC, OHW]
        BB = np.stack([np.broadcast_to(be[int(seg_expert[c, si])], (128, O))
                       for si in range(S)]).astype(np.float32)
        WSL = np.ascontiguousarray(wgt[c].reshape(T_pc, 128).T)  # [128, T_pc]
        in_maps.append({"xg": XG, "wseg": WSEG, "bb": BB, "wsl": WSL})

    return {"key": (T_pc, sizes), "in_maps": in_maps, "rid": rid}


def combine(plan, outs):
    R = np.concatenate(
        [np.asarray(o).astype(np.float32) for o in outs], axis=0)
    rid = plan["rid"]
    return R[rid[:, 0]] + R[rid[:, 1]]


# ----------------------------------------------------------------------------
# Device program
# ----------------------------------------------------------------------------

def build_program(T_pc, sizes):
    S = len(sizes)
    S_slots = T_pc * 128

    nc = bacc.Bacc("TRN2", target_bir_lowering=False, debug=False,
                   num_devices=N_CORES)

    xgd = nc.dram_tensor("xg", [128, T_pc, KC, 128], BF16, kind="ExternalInput").ap()
    wsegd = nc.dram_tensor("wseg", [S, OH, 128, KC, OHW], BF16, kind="ExternalInput").ap()
    bbd = nc.dram_tensor("bb", [S, 128, O], F32, kind="ExternalInput").ap()
    wsld = nc.dram_tensor("wsl", [128, T_pc], F32, kind="ExternalInput").ap()
    outd = nc.dram_tensor("outd", [S_slots, O], BF16, kind="ExternalOutput").ap()

    AF = mybir.ActivationFunctionType
    ALU = mybir.AluOpType

    KG = 4          # weight k-chunks per DMA: first matmul starts after 512KB
    NG = KC // KG   # 4 chunk-tiles per (s, oh) weight block

    with tile.TileContext(nc) as tc:
        with (
            tc.tile_pool(name="singles", bufs=1) as singles,
            tc.tile_pool(name="wpool", bufs=2 * NG) as wpool,
            tc.tile_pool(name="bpool", bufs=min(S, 2)) as bpool,
            tc.tile_pool(name="mpsum", bufs=8, space="PSUM") as mpsum,
            tc.tile_pool(name="rowp", bufs=max(sizes) + (2 if S > 1 else 0)) as rowp,
            tc.tile_pool(name="smallp", bufs=max(sizes) + 4) as smallp,
        ):
            # DMA issue order: the first matmul group needs only xg tile 0 and
            # weight chunk (s0, oh0, g0). Keep the critical set (xg0/xg1,
            # seg-0 bias, wsl) on the scalar ring; the remaining xg tiles are
            # queued on the sync ring BEHIND the first weight block's chunks
            # so they don't steal bandwidth from the startup critical path.
            xg_sb = {}
            for t in range(min(2, T_pc)):
                xg_sb[t] = singles.tile([128, KC, 128], BF16, name=f"xg{t}")
                nc.scalar.dma_start(out=xg_sb[t], in_=xgd[:, t])
            bias0_sb = bpool.tile([128, O], F32, tag="bias", name="bias0")
            nc.scalar.dma_start(out=bias0_sb, in_=bbd[0])
            wsl_sb = singles.tile([128, T_pc], F32)
            nc.scalar.dma_start(out=wsl_sb, in_=wsld)
            for t in range(2, T_pc):
                xg_sb[t] = singles.tile([128, KC, 128], BF16, name=f"xg{t}")

            rowbufs, sums = {}, {}
            t0 = 0
            for s in range(S):
                tlist = list(range(t0, t0 + sizes[s]))
                t0 += sizes[s]
                bias_sb = None
                for oh in range(OH):
                    wt = []
                    for g in range(NG):
                        w = wpool.tile([128, KG, OHW], BF16, tag="wsb")
                        nc.sync.dma_start(out=w, in_=wsegd[s, oh, :, g * KG:(g + 1) * KG])
                        wt.append(w)
                    if s == 0 and oh == 0:
                        for t in range(2, T_pc):
                            nc.sync.dma_start(out=xg_sb[t], in_=xgd[:, t])
                    if oh == 0:
                        if s == 0:
                            bias_sb = bias0_sb
                        else:
                            bias_sb = bpool.tile([128, O], F32, tag="bias")
                            nc.sync.dma_start(out=bias_sb, in_=bbd[s])
                    for t in tlist:
                        if oh == 0:
                            rowbufs[t] = rowp.tile([128, O], BF16, tag="rowbuf",
                                                   name=f"rowbuf{t}")
                            sums[t] = smallp.tile([128, OH], F32, tag="sums",
                                                  name=f"sums{t}")
                        ps = mpsum.tile([128, OHW], F32)
                        for k in range(KC):
                            g, r = divmod(k, KG)
                            nc.tensor.matmul(ps, lhsT=xg_sb[t][:, k, :],
                                             rhs=wt[g][:, r, :],
                                             start=(k == 0), stop=(k == KC - 1))
                        seg_row = rowbufs[t][:, oh * OHW:(oh + 1) * OHW]
                        nc.vector.tensor_tensor(
                            out=seg_row, in0=ps,
                            in1=bias_sb[:, oh * OHW:(oh + 1) * OHW], op=ALU.add)
                        nc.vector.tensor_scalar_max(seg_row, seg_row, 0.0)
                        nc.scalar.activation(seg_row, seg_row, AF.Exp,
                                             accum_out=sums[t][:, oh:oh + 1])
                for t in tlist:
                    stot = smallp.tile([128, 1], F32, tag="stot")
                    nc.vector.tensor_reduce(stot, sums[t],
                                            axis=mybir.AxisListType.X, op=ALU.add)
                    nc.vector.reciprocal(stot, stot)
                    scl = smallp.tile([128, 1], F32, tag="scl")
                    nc.vector.tensor_tensor(out=scl, in0=stot,
                                            in1=wsl_sb[:, t:t + 1], op=ALU.mult)
                    nc.vector.tensor_scalar_mul(rowbufs[t], rowbufs[t], scl[:, :1])
                    nc.scalar.dma_start(out=outd[t * 128:(t + 1) * 128, :],
                                        in_=rowbufs[t][:])
                    del rowbufs[t], sums[t]

    nc.compile()
    return nc


_PROGRAM_CACHE = {}


def _get_program(key):
    if key not in _PROGRAM_CACHE:
        _PROGRAM_CACHE[key] = build_program(*key)
    return _PROGRAM_CACHE[key]


def kernel(inputs, We, be, Wg, bg, top_x):
    assert int(top_x) == 2, "kernel specialized for top_x=2"
    plan = make_plan(inputs, We, be, Wg, bg)
    nc = _get_program(plan["key"])
    res = run_bass_kernel_spmd(nc, plan["in_maps"], list(range(N_CORES)))
    return combine(plan, [r["outd"] for r in res.results])


# revision 23
# speedup vs baseline: 1.0118x; 1.0118x over previous
"""Trainium2 Bass kernel for top-2-of-8 MoE routing (nn_MoETopX).

Reference semantics (computed densely there, routed here):
    gate_logits = x @ Wg + bg                       # [N, 8]
    top_vals, top_idx = top_k(gate_logits, 2)
    w = softmax(softmax(top_vals))                  # double softmax, [N, 2]
    h_e = x @ We[e] + be[e]       for the 2 selected experts per token
    y_e = softmax(relu(h_e), axis=-1)
    out = sum_e w_e * y_e                           # [N, 2048]

Strategy: expert-major sharding of the routed (token, expert) slot list.
The 16384 routed slots are grouped by expert into 128-row tiles and the
tiles are bin-packed onto 8 cores as S weight "segments" per core (every
core runs the identical program: same tile count T_PC and same per-segment
tile counts; which expert a segment is bound to is per-core input data).
Each core therefore loads only the S expert weight matrices its segments
need (~2x8MB bf16) instead of all 8 (64MB) -- the previous data-parallel
version was DMA-bound on exactly that weight traffic.

Per core the device program is a dense pipeline with no indirect DMA:
  1. load routed activations xg (gathered/transposed on host, bf16),
  2. per (segment, out-quarter): stream the weight block, run 16-chunk
     bf16 matmuls into PSUM for each 128-slot tile,
  3. h+bias on DVE (bias pre-broadcast to 128 partitions by the host),
     relu (+cast to bf16) on DVE, exp with fused row-sum accum on ACT,
  4. per tile: scale rows by combine_weight/rowsum, DMA out as bf16.

Host python does the routing metadata (argsort top-2, double-softmax
combine weights, bin packing, gathers/layout) and the final 2-rows-per-
token sum; all expert-matmul FLOPs (99.8% of model FLOPs) run on device.
"""

import numpy as np
import ml_dtypes

import concourse.bass as bass  # noqa: F401  (kept for parity with bass_utils expectations)
import concourse.tile as tile
from concourse import bacc, mybir
from concourse.bass_utils import run_bass_kernel_spmd

F32 = mybir.dt.float32
BF16 = mybir.dt.bfloat16

N_CORES = 8
N_TOKENS = 8192
D = 2048
O = 2048
E = 8
KC = D // 128   # 16 contraction chunks
OH = 4          # output-dim quarters (one 2KB PSUM bank per matmul)
OHW = O // OH   # 512


# ----------------------------------------------------------------------------
# Host-side routing + packing
# ----------------------------------------------------------------------------

def _softmax2(v):
    m = v.max(axis=1, keepdims=True)
    e = np.exp(v - m)
    return e / e.sum(axis=1, keepdims=True)


def _route(x, Wg, bg):
    """fp32 gate, top-2 (matches jax.lax.top_k tie order), double softmax."""
    logits = x @ Wg + bg
    order = np.argsort(-logits, axis=1, kind="stable")
    top2 = order[:, :2].astype(np.int32)
    v = np.take_along_axis(logits, top2, axis=1)
    w = _softmax2(_softmax2(v))
    return top2, w.astype(np.float32)


def _size_candidates(T_pc):
    """Per-core segment tile-count vectors to try, fewest segments first
    (fewer segments = fewer expert weight blocks DMAd per core)."""
    out = []
    for S in (1, 2, 3, 4):
        if T_pc >= S:
            base, r = divmod(T_pc, S)
            out.append(tuple([base + 1] * r + [base] * (S - r)))
    return out


def _try_assign(tiles_e, sizes):
    """Greedily assign each expert a multiset of unit sizes (units = 8 copies
    of `sizes`) covering tiles_e with minimal waste. Returns {e: {size: n}}
    or None."""
    pool = {}
    for sz in sizes:
        pool[sz] = pool.get(sz, 0) + N_CORES
    szs = sorted(pool)
    assign = {}
    for e in sorted(range(E), key=lambda e: -tiles_e[e]):
        need = int(tiles_e[e])
        if need == 0:
            assign[e] = {}
            continue
        best = None
        counts = [range(pool[s] + 1) for s in szs]
        import itertools
        for combo in itertools.product(*counts):
            tot = sum(c * s for c, s in zip(combo, szs))
            if tot < need:
                continue
            cand = (tot - need, sum(combo), combo)
            if best is None or cand[:2] < best[:2]:
                best = cand
        if best is None:
            return None
        assign[e] = {s: c for s, c in zip(szs, best[2]) if c}
        for s, c in assign[e].items():
            pool[s] -= c
    return assign


def _plan_structure(tiles_e):
    TT = int(tiles_e.sum())
    t_min = max(1, -(-TT // N_CORES))
    for T_pc in range(t_min, t_min + 8):
        for sizes in _size_candidates(T_pc):
            asg = _try_assign(tiles_e, sizes)
            if asg is not None:
                return T_pc, sizes, asg
    raise RuntimeError("packing failed")


def _wseg_of(cache, We, e):
    if e not in cache:
        cache[e] = np.ascontiguousarray(
            We[e].reshape(KC, 128, OH, OHW).transpose(2, 1, 0, 3)
        ).astype(ml_dtypes.bfloat16)
    return cache[e]


def make_plan(x, We, be, Wg, bg):
    x = np.asarray(x, dtype=np.float32)
    We = np.asarray(We, dtype=np.float32)
    be = np.asarray(be, dtype=np.float32)
    Wg = np.asarray(Wg, dtype=np.float32)
    bg = np.asarray(bg, dtype=np.float32)

    top2, w = _route(x, Wg, bg)
    cnt = np.bincount(top2.ravel(), minlength=E)
    tiles_e = np.ceil(cnt / 128).astype(int)
    T_pc, sizes, assign = _plan_structure(tiles_e)
    S = len(sizes)
    S_slots = T_pc * 128
    seg_start = np.concatenate([[0], np.cumsum(sizes)])[:-1]

    # instantiate units: per size, free (core, seg) list
    free = {}
    for c in range(N_CORES):
        for si, sz in enumerate(sizes):
            free.setdefault(sz, []).append((c, si))
    expert_units = {}
    for e in range(E):
        expert_units[e] = []
        for sz, k in sorted(assign[e].items(), reverse=True):
            for _ in range(k):
                expert_units[e].append((free[sz].pop(), sz))

    tok = np.zeros((N_CORES, S_slots), np.int32)
    wgt = np.zeros((N_CORES, S_slots), np.float32)
    seg_expert = np.zeros((N_CORES, S), np.int32)
    rows_tok, rows_gid = [], []
    for e in range(E):
        sel_t, sel_r = np.where(top2 == e)
        we_vals = w[sel_t, sel_r]
        off = 0
        for (c, si), sz in expert_units[e]:
            seg_expert[c, si] = e
            n = min(sz * 128, len(sel_t) - off)
            if n <= 0:
                continue
            base = seg_start[si] * 128
            tok[c, base:base + n] = sel_t[off:off + n]
            wgt[c, base:base + n] = we_vals[off:off + n]
            gid0 = c * S_slots + base
            rows_gid.append(np.arange(gid0, gid0 + n, dtype=np.int64))
            rows_tok.append(sel_t[off:off + n])
            off += n
        assert off == len(sel_t), (e, off, len(sel_t))

    at = np.concatenate(rows_tok)
    ag = np.concatenate(rows_gid)
    rid = ag[np.argsort(at, kind="stable")].reshape(N_TOKENS, 2)

    in_maps = []
    wcache = {}
    for c in range(N_CORES):
        A = x[tok[c]]                                     # [S_slots, D]
        XG = np.ascontiguousarray(
            A.reshape(T_pc, 128, KC, 128).transpose(3, 0, 2, 1)
        ).astype(ml_dtypes.bfloat16)                      # [128, T_pc, KC, 128]
        WSEG = np.stack([_wseg_of(wcache, We, int(seg_expert[c, si]))
                         for si in range(S)])             # [S, OH, 128, KC, OHW]
        BB = np.stack([np.broadcast_to(be[int(seg_expert[c, si])], (128, O))
                       for si in range(S)]).astype(np.float32)
        WSL = np.ascontiguousarray(wgt[c].reshape(T_pc, 128).T)  # [128, T_pc]
        in_maps.append({"xg": XG, "wseg": WSEG, "bb": BB, "wsl": WSL})

    return {"key": (T_pc, sizes), "in_maps": in_maps, "rid": rid}


def combine(plan, outs):
    R = np.concatenate(
        [np.asarray(o).astype(np.float32) for o in outs], axis=0)
    rid = plan["rid"]
    return R[rid[:, 0]] + R[rid[:, 1]]


# ----------------------------------------------------------------------------
# Device program
# ----------------------------------------------------------------------------

def build_program(T_pc, sizes):
    S = len(sizes)
    S_slots = T_pc * 128

    nc = bacc.Bacc("TRN2", target_bir_lowering=False, debug=False,
                   num_devices=N_CORES)

    xgd = nc.dram_tensor("xg", [128, T_pc, KC, 128], BF16, kind="ExternalInput").ap()
    wsegd = nc.dram_tensor("wseg", [S, OH, 128, KC, OHW], BF16, kind="ExternalInput").ap()
    bbd = nc.dram_tensor("bb", [S, 128, O], F32, kind="ExternalInput").ap()
    wsld = nc.dram_tensor("wsl", [128, T_pc], F32, kind="ExternalInput").ap()
    outd = nc.dram_tensor("outd", [S_slots, O], BF16, kind="ExternalOutput").ap()

    AF = mybir.ActivationFunctionType
    ALU = mybir.AluOpType

    KG = 4          # weight k-chunks per DMA: first matmul starts after 512KB
    NG = KC // KG   # 4 chunk-tiles per (s, oh) weight block

    with tile.TileContext(nc) as tc:
        with (
            tc.tile_pool(name="singles", bufs=1) as singles,
            tc.tile_pool(name="wpool", bufs=2 * NG) as wpool,
            tc.tile_pool(name="bpool", bufs=min(S, 2)) as bpool,
            tc.tile_pool(name="mpsum", bufs=6, space="PSUM") as mpsum,
            tc.tile_pool(name="rowp", bufs=max(sizes) + (2 if S > 1 else 0)) as rowp,
            tc.tile_pool(name="smallp", bufs=max(sizes) + 4) as smallp,
        ):
            # DMA issue order: the first matmul group needs only xg tile 0 and
            # weight chunk (s0, oh0, g0); keep those at the head of their rings.
            xg_sb = {}
            for t in range(2):
                xg_sb[t] = singles.tile([128, KC, 128], BF16, name=f"xg{t}")
                nc.scalar.dma_start(out=xg_sb[t], in_=xgd[:, t])
            wsl_sb = singles.tile([128, T_pc], F32)
            nc.scalar.dma_start(out=wsl_sb, in_=wsld)
            for t in range(2, T_pc):
                xg_sb[t] = singles.tile([128, KC, 128], BF16, name=f"xg{t}")
                nc.scalar.dma_start(out=xg_sb[t], in_=xgd[:, t])

            rowbufs, sums = {}, {}
            t0 = 0
            for s in range(S):
                tlist = list(range(t0, t0 + sizes[s]))
                t0 += sizes[s]
                bias_sb = None
                for oh in range(OH):
                    wt = []
                    for g in range(NG):
                        w = wpool.tile([128, KG, OHW], BF16, tag="wsb")
                        nc.sync.dma_start(out=w, in_=wsegd[s, oh, :, g * KG:(g + 1) * KG])
                        wt.append(w)
                    if oh == 0:
                        # bias lands while the first matmul groups run
                        bias_sb = bpool.tile([128, O], F32, tag="bias")
                        nc.sync.dma_start(out=bias_sb, in_=bbd[s])
                    for t in tlist:
                        if oh == 0:
                            rowbufs[t] = rowp.tile([128, O], BF16, tag="rowbuf",
                                                   name=f"rowbuf{t}")
                            sums[t] = smallp.tile([128, OH], F32, tag="sums",
                                                  name=f"sums{t}")
                        ps = mpsum.tile([128, OHW], F32)
                        for k in range(KC):
                            g, r = divmod(k, KG)
                            nc.tensor.matmul(ps, lhsT=xg_sb[t][:, k, :],
                                             rhs=wt[g][:, r, :],
                                             start=(k == 0), stop=(k == KC - 1))
                        seg_row = rowbufs[t][:, oh * OHW:(oh + 1) * OHW]
                        nc.vector.tensor_tensor(
                            out=seg_row, in0=ps,
                            in1=bias_sb[:, oh * OHW:(oh + 1) * OHW], op=ALU.add)
                        nc.vector.tensor_scalar_max(seg_row, seg_row, 0.0)
                        nc.scalar.activation(seg_row, seg_row, AF.Exp,
                                             accum_out=sums[t][:, oh:oh + 1])
                for t in tlist:
                    stot = smallp.tile([128, 1], F32, tag="stot")
                    nc.vector.tensor_reduce(stot, sums[t],
                                            axis=mybir.AxisListType.X, op=ALU.add)
                    nc.vector.reciprocal(stot, stot)
                    scl = smallp.tile([128, 1], F32, tag="scl")
                    nc.vector.tensor_tensor(out=scl, in0=stot,
                                            in1=wsl_sb[:, t:t + 1], op=ALU.mult)
                    nc.vector.tensor_scalar_mul(rowbufs[t], rowbufs[t], scl[:, :1])
                    nc.scalar.dma_start(out=outd[t * 128:(t + 1) * 128, :],
                                        in_=rowbufs[t][:])
                    del rowbufs[t], sums[t]

    nc.compile()
    return nc


_PROGRAM_CACHE = {}


def _get_program(key):
    if key not in _PROGRAM_CACHE:
        _PROGRAM_CACHE[key] = build_program(*key)
    return _PROGRAM_CACHE[key]


def kernel(inputs, We, be, Wg, bg, top_x):
    assert int(top_x) == 2, "kernel specialized for top_x=2"
    plan = make_plan(inputs, We, be, Wg, bg)
    nc = _get_program(plan["key"])
    res = run_bass_kernel_spmd(nc, plan["in_maps"], list(range(N_CORES)))
    return combine(plan, [r["outd"] for r in res.results])
